# revision 18
# baseline (speedup 1.0000x reference)
"""CenterLoss on 8 Trainium2 NeuronCores (Bass/Tile).

loss = clip(distmat * onehot(labels), 1e-12, 1e12).sum() / B
     = (sum_i ||x_i - c_{y_i}||^2 + B*(C-1)*1e-12) / B        (clip inactive:
       d_i in [333, 712] for these input stats)

Sharding strategy: instead of splitting the batch by position (which forces a
per-sample indirect-DMA gather of center rows -- SWDGE descriptor generation is
serial on GpSimd at ~8.4ns/row = a ~34us floor for 4096 rows/core), samples are
routed on the host to the core that owns their label's 128-class group
(g = label >> 7).  The loss is a pure sum over samples, so any sample->core
assignment is valid sharding.  Each core then only touches 128 distinct
classes and the gather becomes a dense one-hot matmul:

  sum_i ||x_i - c_{y_i}||^2 = sum_i ||x_i||^2 + sum_c n_c ||c_c||^2
                              - 2 sum_c s_c . c_c,   s = H^T X  (H one-hot)

x is sent bf16 (2e-2 tolerance; measured end error ~2e-6), laid out
partition-major so chunks stream as contiguous multi-KB descriptors at HBM
line rate.  Per 128-sample tile the DVE builds H[sample, class] with one
iota/is_equal tensor_scalar (bf16, labels as per-partition f32 scalar column)
and the PE accumulates s = H^T X (bf16, 1 cycle/row) into one PSUM bank.
sum||x||^2 runs off the critical path: ScalarE activation(Square, accum) on
half the chunks, GpSimd scalar_tensor_tensor(x*x, accum) on the other half.
Trace-driven details: DMA triggers cost ~650ns each serially per engine, so
they are split between the Sync and Activation HWDGE queues and issued before
any compute; chunk sizes ramp (2,4,6,8...) so the first matmul starts early
and the last chunk drains fast; centers/counts/labels ship as one aux tensor
(one trigger); epilogue constants (c_sq, counts*c_sq) compute during the main
loop; the tail is one fused scalar_tensor_tensor((-2*s) * c, accum) plus a
single [128,3] ones-matmul whose 3 partial sums are added on the host along
with the 8-core reduction (the sanctioned scalar all-reduce).

Pads: shards pad to a common tile count with rows equal to centers[g*128]
labelled class 0, so x_sq + c_sq - 2 x.c == 0 (only bf16 rounding residue,
~1e-7 relative).
"""

import numpy as np

BATCH, NUM_CLASSES, FEATURE_DIM = 32768, 1024, 256
N_CORES = 8
GROUP_CLASSES = NUM_CLASSES // N_CORES  # 128
P = 128
CLAMP_MIN = 1e-12

_CACHE: dict = {}


def _chunk_sizes(nt: int) -> list[int]:
    """Ramp chunk sizes: small first chunk (early first matmul), small last
    chunk (fast drain), 8-tile body."""
    if nt <= 4:
        return [nt]
    sizes, rem = [], nt
    for s in (2, 4, 6):
        if rem >= s + 4:
            sizes.append(s)
            rem -= s
    while rem > 10:
        sizes.append(8)
        rem -= 8
    if rem > 4:
        sizes.extend([rem - 2, 2])
    else:
        sizes.append(rem)
    return sizes


def _build_nc(nt: int):
    import concourse.bacc as bacc
    import concourse.tile as tile
    from concourse import mybir

    f32 = mybir.dt.float32
    bf16 = mybir.dt.bfloat16

    sizes = _chunk_sizes(nt)
    n_chunks = len(sizes)
    offs = [sum(sizes[:i]) for i in range(n_chunks)]
    AUX_W = FEATURE_DIM + 1 + nt  # centers | counts | labels

    nc = bacc.Bacc("TRN2", target_bir_lowering=False, debug=False)

    # x partition-major: x_d[p*nt + j, :] -> partition p, tile j
    x_d = nc.dram_tensor("x", [nt * P, FEATURE_DIM], bf16, kind="ExternalInput")
    aux_d = nc.dram_tensor("aux", [P, AUX_W], f32, kind="ExternalInput")
    out_d = nc.dram_tensor("out", [3, 1], f32, kind="ExternalOutput")

    x_v = x_d.rearrange("(p j) e -> p j e", p=P)

    with tile.TileContext(nc) as tc:
        with (
            tc.tile_pool(name="xdata", bufs=n_chunks) as xpool,
            tc.tile_pool(name="hbuf", bufs=8) as hpool,
            tc.tile_pool(name="scratch", bufs=2) as spool,
            tc.tile_pool(name="gscratch", bufs=2) as gpool,
            tc.tile_pool(name="single", bufs=1) as single,
            tc.tile_pool(name="psum", bufs=2, space="PSUM") as psum,
        ):
            # ---- DMA triggers first: split across the two HWDGE queues ----
            aux_sb = single.tile([P, AUX_W], f32)
            nc.scalar.dma_start(out=aux_sb[:], in_=aux_d[:, :])
            chunk_tiles = []
            for c in range(n_chunks):
                x_t = xpool.tile([P, sizes[c], FEATURE_DIM], bf16, tag="x")
                eng = nc.sync if c % 2 == 0 else nc.scalar
                eng.dma_start(out=x_t[:], in_=x_v[:, offs[c] : offs[c] + sizes[c], :])
                chunk_tiles.append(x_t)

            cen_sb = aux_sb[:, 0:FEATURE_DIM]
            cnt_sb = aux_sb[:, FEATURE_DIM : FEATURE_DIM + 1]
            lab_sb = aux_sb[:, FEATURE_DIM + 1 : AUX_W]

            # ---- constants ----
            iota_row = single.tile([P, P], bf16)
            nc.gpsimd.iota(
                iota_row[:],
                pattern=[[1, P]],
                base=0,
                channel_multiplier=0,
                allow_small_or_imprecise_dtypes=True,
            )
            ones = single.tile([P, 1], f32)
            nc.vector.memset(ones[:], 1.0)

            # combo columns: [xsq_sum | n*csq | -2*cross]
            combo = single.tile([P, 3], f32)

            # ---- early epilogue precomputes (run during main loop) ----
            cen_sq_scr = single.tile([P, FEATURE_DIM], f32)
            csq = single.tile([P, 1], f32)
            nc.scalar.activation(
                out=cen_sq_scr[:],
                in_=cen_sb,
                func=mybir.ActivationFunctionType.Square,
                accum_out=csq[:],
            )
            nc.vector.tensor_tensor(
                out=combo[:, 1:2], in0=cnt_sb, in1=csq[:], op=mybir.AluOpType.mult
            )

            # ---- main loop: H one-hot (DVE) + s += H^T x (PE), x^2 interleaved ----
            ps_s = psum.tile([P, FEATURE_DIM], f32, space="PSUM")
            xsq = single.tile([P, n_chunks], f32)
            for c in range(n_chunks):
                for j in range(sizes[c]):
                    t = offs[c] + j
                    h_t = hpool.tile([P, P], bf16, tag="h")
                    nc.vector.tensor_scalar(
                        out=h_t[:],
                        in0=iota_row[:],
                        scalar1=lab_sb[:, t : t + 1],
                        scalar2=None,
                        op0=mybir.AluOpType.is_equal,
                    )
                    nc.tensor.matmul(
                        out=ps_s[:],
                        lhsT=h_t[:],
                        rhs=chunk_tiles[c][:, j, :],
                        start=(t == 0),
                        stop=(t == nt - 1),
                    )
                # x^2 row-accumulate for this chunk, off the critical path
                flat = chunk_tiles[c][:].rearrange("p j e -> p (j e)")
                if c % 2 == 0:
                    scr = spool.tile([P, sizes[c] * FEATURE_DIM], bf16, tag="sq")
                    nc.scalar.activation(
                        out=scr[:],
                        in_=flat,
                        func=mybir.ActivationFunctionType.Square,
                        accum_out=xsq[:, c : c + 1],
                    )
                else:
                    scr = gpool.tile([P, sizes[c] * FEATURE_DIM], bf16, tag="gq")
                    nc.vector.scalar_tensor_tensor(
                        out=scr[:],
                        in0=flat,
                        scalar=1.0,
                        in1=flat,
                        op0=mybir.AluOpType.mult,
                        op1=mybir.AluOpType.mult,
                        accum_out=xsq[:, c : c + 1],
                    )

            # ---- epilogue ----
            nc.vector.reduce_sum(
                out=combo[:, 0:1], in_=xsq[:], axis=mybir.AxisListType.X
            )
            # (-2*s) . c, row-accumulated in one fused op
            prod_scr = single.tile([P, FEATURE_DIM], f32)
            nc.vector.scalar_tensor_tensor(
                out=prod_scr[:],
                in0=ps_s[:],
                scalar=-2.0,
                in1=cen_sb,
                op0=mybir.AluOpType.mult,
                op1=mybir.AluOpType.mult,
                accum_out=combo[:, 2:3],
            )
            ps_f = psum.tile([3, 1], f32, space="PSUM")
            nc.tensor.matmul(
                out=ps_f[:], lhsT=combo[:], rhs=ones[:], start=True, stop=True
            )
            res = single.tile([3, 1], f32)
            nc.vector.tensor_copy(out=res[:], in_=ps_f[:])
            nc.sync.dma_start(out=out_d[:, :], in_=res[:])

    nc.finalize()
    return nc


def kernel(x: np.ndarray, centers: np.ndarray, labels: np.ndarray) -> np.ndarray:
    from concourse import bass_utils, mybir

    bf16_np = mybir.dt.np(mybir.dt.bfloat16)
    x = np.ascontiguousarray(np.asarray(x, dtype=np.float32))
    centers = np.ascontiguousarray(np.asarray(centers, dtype=np.float32))
    lab = np.asarray(labels).astype(np.int64).ravel()

    grp = lab >> 7
    order = np.argsort(grp, kind="stable")
    gcounts = np.bincount(grp, minlength=N_CORES)
    nt = max(1, int(-(-int(gcounts.max()) // P)))  # ceil(max_shard/128) tiles
    pad = nt * P
    starts = np.concatenate(([0], np.cumsum(gcounts)))

    key = ("nc", nt)
    if key not in _CACHE:
        _CACHE[key] = _build_nc(nt)
    nc = _CACHE[key]

    in_maps = []
    for c in range(N_CORES):
        idx = order[starts[c] : starts[c + 1]]
        n = idx.shape[0]
        xc = np.empty((pad, FEATURE_DIM), dtype=bf16_np)
        # partition-major layout: row p*nt + j -> partition p, tile j
        xc[:n] = x[idx]
        xc[n:] = centers[c * GROUP_CLASSES]  # pad rows: d ~= 0 (bf16 residue)
        lab_loc = np.zeros(pad, dtype=np.int64)
        lab_loc[:n] = lab[idx] - c * GROUP_CLASSES
        cnt = np.bincount(lab_loc, minlength=GROUP_CLASSES).astype(np.float32)
        aux = np.empty((P, FEATURE_DIM + 1 + nt), dtype=np.float32)
        aux[:, 0:FEATURE_DIM] = centers[
            c * GROUP_CLASSES : (c + 1) * GROUP_CLASSES
        ]
        aux[:, FEATURE_DIM] = cnt
        aux[:, FEATURE_DIM + 1 :] = lab_loc.reshape(P, nt).astype(np.float32)
        in_maps.append({"x": xc, "aux": aux})

    rr = bass_utils.run_bass_kernel_spmd(nc, in_maps, list(range(N_CORES)))
    _CACHE["last_results"] = rr

    total = sum(float(r["out"][i, 0]) for r in rr.results for i in range(3))
    loss = (total + BATCH * (NUM_CLASSES - 1) * CLAMP_MIN) / BATCH
    return np.asarray(loss, dtype=np.float32)
